# revision 3
# baseline (speedup 1.0000x reference)
"""Trainium2 Bass kernel for the CfC cell (nn_CfCCell), data-parallel on 8 cores.

Math (per row):
    ff1 = gelu(x_cat @ W_ff1.T + b_ff1)          x_cat = [x, hx]
    ff2 = gelu(ff1 @ W_ff2.T + b_ff2)
    t   = sigmoid(ff2 @ (W_ta+W_tb).T + b_ta+b_tb)      (TS == 1.0)
    ic  = gelu(x @ W_in.T + b_in + input_b)
    rc  = gelu(hx @ W_r.T + r_b)
    out = hx + t * (ic + rc - hx)

v2 design notes:
  * batch sharded 8 ways; all activations feature-major ([feat, batch]).
  * x/hx are cast to bf16 AND transposed on the HOST, so the device loads
    feature-major bf16 directly (halves HBM traffic, removes all PE
    transposes).  The output is stored feature-major bf16 and
    transposed/upcast on the host.
  * PSUM is split 4+4 banks: matmuls fill one [128,2048] f32 group while
    ScalarE drains the other with a single N=2048 ACTIVATE per layer-half
    (ACT is the critical engine at ~148us; this keeps it ~92% efficient).
  * sigmoid via 0.5*tanh(z/2)+0.5 so every ScalarE op lives in the single
    "gelu_and_others" table set (no table reloads).
  * weights stay stationary across the 4 psum banks (k-outer matmul loop)
    to minimize LDWEIGHTS traffic.
"""

from contextlib import ExitStack

import ml_dtypes
import numpy as np

import concourse.bacc as bacc
import concourse.bass as bass
import concourse.mybir as mybir
import concourse.tile as tile
from concourse import masks
from concourse.bass_utils import run_bass_kernel_spmd

AF = mybir.ActivationFunctionType
ALU = mybir.AluOpType
BF16 = mybir.dt.bfloat16
F32 = mybir.dt.float32
NP_BF16 = ml_dtypes.bfloat16

B, I, H = 131072, 128, 256
N_CORES = 8
B_CORE = B // N_CORES  # 16384
R = 2048               # megatile rows (batch columns per megatile)

# layer order; K = contraction chunks of 128
LAYERS = ("ff1", "ff2", "tab", "ic", "rc")
KCH = {"ff1": 3, "ff2": 2, "tab": 2, "ic": 1, "rc": 2}
W_BASE = {}
_acc = 0
for _l in LAYERS:
    W_BASE[_l] = _acc
    _acc += KCH[_l] * 2
N_WCH = _acc  # 20 weight chunks of [128, 128]
BIAS_COL = {(_l, _m): 2 * _i + _m for _i, _l in enumerate(LAYERS) for _m in range(2)}


def build_nc(b_core: int = B_CORE, r: int = R) -> bass.Bass:
    nm = b_core // r
    assert b_core % r == 0 and r % 1024 == 0

    nc = bacc.Bacc("TRN2")
    in_d = nc.dram_tensor("inT", [3, 128, b_core], BF16, kind="ExternalInput")
    w_d = nc.dram_tensor("wstack", [N_WCH, 128, 128], BF16, kind="ExternalInput")
    b_d = nc.dram_tensor("bstack", [128, 10], F32, kind="ExternalInput")
    out_d = nc.dram_tensor("outT", [2, 128, b_core], BF16, kind="ExternalOutput")

    with tile.TileContext(nc) as tc, ExitStack() as ctx:
        const = ctx.enter_context(tc.tile_pool(name="const", bufs=1))
        w_sb = const.tile([128, N_WCH * 128], BF16)
        nc.sync.dma_start(
            w_sb[:].rearrange("p (c f) -> p c f", c=N_WCH),
            w_d[:].rearrange("c p f -> p c f"))
        b_sb = const.tile([128, 10], F32)
        nc.sync.dma_start(b_sb[:], b_d[:])
        ident = const.tile([128, 128], BF16)
        masks.make_identity(nc, ident[:])

        io = ctx.enter_context(tc.tile_pool(name="io", bufs=2))
        acts = ctx.enter_context(tc.tile_pool(name="acts", bufs=2))
        tmp = ctx.enter_context(tc.tile_pool(name="tmp", bufs=2))
        ps = ctx.enter_context(tc.tile_pool(name="ps", bufs=2, space="PSUM"))

        # HAM warm-up: ~3.5us of dummy PE work while the first loads land, so
        # the first real matmuls run at 2.4 GHz instead of 1.2
        warm = ps.tile([128, 2048], F32, tag="mm")
        for i in range(32):
            nc.tensor.matmul(
                warm[:, (i % 16) * 128:(i % 16 + 1) * 128], ident[:], ident[:])

        def wchunk(layer, k, m):
            ci = W_BASE[layer] + 2 * k + m
            return w_sb[:, ci * 128:(ci + 1) * 128]

        def stage_a(r0, rt, first):
            """Loads + all matmul/activation layers for batch cols [r0,r0+rt)."""
            nj = rt // 512
            in_sb = io.tile([128, 3 * r], BF16, tag="in")
            # feature-major loads, split so the first column block lands early
            ng = 2
            hw_ = rt // ng
            for g in range(ng):
                for c in range(3):
                    nc.gpsimd.dma_start(
                        in_sb[:, c * rt + g * hw_:c * rt + (g + 1) * hw_],
                        in_d[c, :, r0 + g * hw_:r0 + (g + 1) * hw_])

            xT = in_sb[:, 0:rt]
            hxT0 = in_sb[:, rt:2 * rt]
            hxT1 = in_sb[:, 2 * rt:3 * rt]

            def layer_unit(layer, srcs, func, scale, out_tile):
                K = KCH[layer]
                for m in range(2):
                    col = BIAS_COL[(layer, m)]
                    pt = ps.tile([128, 2048], F32, tag="mm")
                    for h in range(rt // 2048):
                        for k in range(K):
                            for j in range(4):
                                sl = slice(h * 2048 + j * 512,
                                           h * 2048 + (j + 1) * 512)
                                nc.tensor.matmul(
                                    pt[:, j * 512:(j + 1) * 512],
                                    wchunk(layer, k, m),
                                    srcs[k][:, sl],
                                    start=(k == 0), stop=(k == K - 1))
                        nc.scalar.activation(
                            out_tile[:, m * rt + h * 2048:m * rt + (h + 1) * 2048],
                            pt[:], func, bias=b_sb[:, col:col + 1], scale=scale)

            # ic/rc first: they only need x/hx, so PE can start before ff1's
            # ACT output exists; ff2/tab then have plenty of slack behind them
            ic = acts.tile([128, 2 * r], BF16, tag="ic")
            layer_unit("ic", [xT], AF.Gelu, 1.0, ic)
            rc = acts.tile([128, 2 * r], BF16, tag="rc")
            layer_unit("rc", [hxT0, hxT1], AF.Gelu, 1.0, rc)
            ff1 = acts.tile([128, 2 * r], BF16, tag="ff1")
            layer_unit("ff1", [xT, hxT0, hxT1], AF.Gelu, 1.0, ff1)
            ff2 = acts.tile([128, 2 * r], BF16, tag="ff2")
            layer_unit("ff2", [ff1[:, 0:rt], ff1[:, rt:2 * rt]], AF.Gelu, 1.0, ff2)
            u = acts.tile([128, 2 * r], BF16, tag="u")
            layer_unit("tab", [ff2[:, 0:rt], ff2[:, rt:2 * rt]], AF.Tanh, 0.5, u)
            return {"r0": r0, "rt": rt, "in_sb": in_sb, "u": u, "ic": ic,
                    "rc": rc}

        def stage_b(st):
            """Combine on DVE, store feature-major."""
            r0, rt = st["r0"], st["rt"]
            in_sb, u, ic, rc = st["in_sb"], st["u"], st["ic"], st["rc"]
            o_sb = io.tile([128, 2 * r], BF16, tag="o")
            # out = hx + t*(ic+rc-hx);  t = 0.5*u + 0.5
            for m in range(2):
                msl = slice(m * rt, (m + 1) * rt)
                hxm = in_sb[:, (1 + m) * rt:(2 + m) * rt]
                ti = tmp.tile([128, r], BF16, tag="ti")
                ti = ti[:, 0:rt]
                nc.vector.tensor_scalar(ti, u[:, msl], 0.5, 0.5, ALU.mult, ALU.add)
                s = tmp.tile([128, r], BF16, tag="s")
                s = s[:, 0:rt]
                nc.vector.tensor_add(s, ic[:, msl], rc[:, msl])
                d = tmp.tile([128, r], BF16, tag="d")
                d = d[:, 0:rt]
                nc.vector.tensor_sub(d, s, hxm)
                p = tmp.tile([128, r], BF16, tag="p")
                p = p[:, 0:rt]
                nc.vector.tensor_mul(p, ti, d)
                nc.vector.tensor_add(o_sb[:, msl], p, hxm)
                nc.gpsimd.dma_start(
                    out_d[m, :, r0:r0 + rt], o_sb[:, msl])

        # software pipeline: defer each megatile's combine/store until after
        # the next megatile's matmul work is queued, so PE/ACT never sit
        # behind the DVE tail (keeps HAM warm across boundaries).
        prev = None
        r0 = 0
        for _ in range(nm):
            st = stage_a(r0, r, first=(r0 == 0))
            r0 += r
            if prev is not None:
                stage_b(prev)
            prev = st
        stage_b(prev)
    nc.finalize()
    return nc


_NC_CACHE: dict = {}


def _get_nc(b_core: int, r: int) -> bass.Bass:
    key = (b_core, r)
    if key not in _NC_CACHE:
        _NC_CACHE[key] = build_nc(b_core, r)
    return _NC_CACHE[key]


def _prep_host(W_ff1, b_ff1, W_ff2, b_ff2, W_ta, b_ta, W_tb, b_tb,
               W_in, b_in, input_b, W_r, r_b):
    f32 = lambda a: np.asarray(a, dtype=np.float32)
    weights = {
        "ff1": f32(W_ff1),
        "ff2": f32(W_ff2),
        "tab": f32(W_ta) + f32(W_tb),
        "ic": f32(W_in),
        "rc": f32(W_r),
    }
    biases = {
        "ff1": f32(b_ff1),
        "ff2": f32(b_ff2),
        "tab": 0.5 * (f32(b_ta) + f32(b_tb)),
        "ic": f32(b_in) + f32(input_b),
        "rc": f32(r_b),
    }
    wstack = np.zeros([N_WCH, 128, 128], dtype=NP_BF16)
    for layer in LAYERS:
        W = weights[layer]
        for k in range(KCH[layer]):
            for m in range(2):
                ci = W_BASE[layer] + 2 * k + m
                wstack[ci] = np.ascontiguousarray(
                    W[m * 128:(m + 1) * 128, k * 128:(k + 1) * 128].T
                ).astype(NP_BF16)
    bstack = np.zeros([128, 10], dtype=np.float32)
    for li, layer in enumerate(LAYERS):
        for m in range(2):
            bstack[:, 2 * li + m] = biases[layer][m * 128:(m + 1) * 128]
    return wstack, bstack


def _run(inputs: dict, b_core: int = B_CORE, r: int = R, n_cores: int = N_CORES,
         **run_kwargs):
    x = np.asarray(inputs["x"], dtype=np.float32)
    hx = np.asarray(inputs["hx"], dtype=np.float32)
    wstack, bstack = _prep_host(
        inputs["W_ff1"], inputs["b_ff1"], inputs["W_ff2"], inputs["b_ff2"],
        inputs["W_ta"], inputs["b_ta"], inputs["W_tb"], inputs["b_tb"],
        inputs["W_in"], inputs["b_in"], inputs["input_b"], inputs["W_r"],
        inputs["r_b"])
    nc = _get_nc(b_core, r)
    xb = x.astype(NP_BF16)
    hxb = hx.astype(NP_BF16)
    in_maps = []
    for c in range(n_cores):
        sl = slice(c * b_core, (c + 1) * b_core)
        inT = np.empty((3, 128, b_core), dtype=NP_BF16)
        inT[0] = xb[sl].T
        inT[1] = hxb[sl, 0:128].T
        inT[2] = hxb[sl, 128:256].T
        in_maps.append({"inT": inT, "wstack": wstack, "bstack": bstack})
    res = run_bass_kernel_spmd(nc, in_maps, list(range(n_cores)), **run_kwargs)
    outs = []
    for m in res.results:
        o = m["outT"]  # [2, 128, b_core] bf16, feature-major
        outs.append(o.transpose(2, 0, 1).astype(np.float32).reshape(b_core, 256))
    out = np.concatenate(outs, axis=0)
    return out, res


def kernel(**inputs):
    out, _ = _run(inputs)
    return (out, out)


# revision 6
# speedup vs baseline: 1.0471x; 1.0471x over previous
"""Trainium2 Bass kernel for the CfC cell (nn_CfCCell), data-parallel on 8 cores.

Math (per row):
    ff1 = gelu(x_cat @ W_ff1.T + b_ff1)          x_cat = [x, hx]
    ff2 = gelu(ff1 @ W_ff2.T + b_ff2)
    t   = sigmoid(ff2 @ (W_ta+W_tb).T + b_ta+b_tb)      (TS == 1.0)
    ic  = gelu(x @ W_in.T + b_in + input_b)
    rc  = gelu(hx @ W_r.T + r_b)
    out = hx + t * (ic + rc - hx)

v3 design notes:
  * batch sharded 8 ways; all activations feature-major ([feat, batch]).
  * x/hx are cast to bf16 AND transposed on the HOST, so the device loads
    feature-major bf16 directly (halves HBM traffic, removes all PE
    transposes).  The output is stored feature-major bf16 and
    transposed/upcast on the host.
  * ScalarE is the critical engine (~157us of ACTIVATE work): PSUM is split
    4+4 banks, matmuls fill one [128,2048] f32 group while ScalarE drains
    the other with a single N=2048 ACTIVATE per layer-half (~87% efficient).
  * layer-level software pipeline: megatile i's independent layers
    (ic/rc/ff1) are interleaved with megatile i-1's dependent layers
    (ff2/tab) so ScalarE never stalls on the ff1->ff2->tab ACT chain.
  * input loads are prefetched one megatile ahead (in pool bufs=3); output
    stores ride the otherwise-idle SP queue so they never block the
    gpsimd load queue.
  * sigmoid via 0.5*tanh(z/2)+0.5 so every ScalarE op lives in the single
    "gelu_and_others" table set (no table reloads).
  * weights stay stationary across the 4 psum banks (k-outer matmul loop)
    to minimize LDWEIGHTS traffic.
"""

from contextlib import ExitStack

import ml_dtypes
import numpy as np

import concourse.bacc as bacc
import concourse.bass as bass
import concourse.mybir as mybir
import concourse.tile as tile
from concourse import masks
from concourse.bass_utils import run_bass_kernel_spmd

AF = mybir.ActivationFunctionType
ALU = mybir.AluOpType
BF16 = mybir.dt.bfloat16
F32 = mybir.dt.float32
NP_BF16 = ml_dtypes.bfloat16

B, I, H = 131072, 128, 256
N_CORES = 8
B_CORE = B // N_CORES  # 16384
R = 2048               # megatile rows (batch columns per megatile)

# layer order; K = contraction chunks of 128
LAYERS = ("ff1", "ff2", "tab", "ic", "rc")
KCH = {"ff1": 3, "ff2": 2, "tab": 2, "ic": 1, "rc": 2}
W_BASE = {}
_acc = 0
for _l in LAYERS:
    W_BASE[_l] = _acc
    _acc += KCH[_l] * 2
N_WCH = _acc  # 20 weight chunks of [128, 128]
BIAS_COL = {(_l, _m): 2 * _i + _m for _i, _l in enumerate(LAYERS) for _m in range(2)}

# per-megatile emission order: A = this megatile's input-only layers,
# B = previous megatile's ff2/tab (interleaved into A's slots)
SEQ_A = (("ic", 0), ("ic", 1), ("rc", 0), ("rc", 1), ("ff1", 0), ("ff1", 1))
SEQ_B = (("ff2", 0), ("ff2", 1), ("tab", 0), ("tab", 1))


def build_nc(b_core: int = B_CORE, r: int = R) -> bass.Bass:
    nm = b_core // r
    assert b_core % r == 0 and r % 1024 == 0

    nc = bacc.Bacc("TRN2")
    in_d = nc.dram_tensor("inT", [3, 128, b_core], BF16, kind="ExternalInput")
    w_d = nc.dram_tensor("wstack", [N_WCH, 128, 128], BF16, kind="ExternalInput")
    b_d = nc.dram_tensor("bstack", [128, 10], F32, kind="ExternalInput")
    out_d = nc.dram_tensor("outT", [2, 128, b_core], BF16, kind="ExternalOutput")

    with tile.TileContext(nc) as tc, ExitStack() as ctx:
        const = ctx.enter_context(tc.tile_pool(name="const", bufs=1))
        w_sb = const.tile([128, N_WCH * 128], BF16)
        nc.sync.dma_start(
            w_sb[:].rearrange("p (c f) -> p c f", c=N_WCH),
            w_d[:].rearrange("c p f -> p c f"))
        b_sb = const.tile([128, 10], F32)
        nc.sync.dma_start(b_sb[:], b_d[:])
        ident = const.tile([128, 128], BF16)
        masks.make_identity(nc, ident[:])

        inp = ctx.enter_context(tc.tile_pool(name="inp", bufs=3))
        io = ctx.enter_context(tc.tile_pool(name="io", bufs=2))
        acts = ctx.enter_context(tc.tile_pool(name="acts", bufs=2))
        tmp = ctx.enter_context(tc.tile_pool(name="tmp", bufs=2))
        ps = ctx.enter_context(tc.tile_pool(name="ps", bufs=2, space="PSUM"))

        # HAM warm-up: ~3.5us of dummy PE work while the first loads land, so
        # the first real matmuls run at 2.4 GHz instead of 1.2
        warm = ps.tile([128, 2048], F32, tag="mm")
        for i in range(32):
            nc.tensor.matmul(
                warm[:, (i % 16) * 128:(i % 16 + 1) * 128], ident[:], ident[:])
        warm2 = ps.tile([128, 2048], F32, tag="mm")
        for i in range(16):
            nc.tensor.matmul(
                warm2[:, (i % 16) * 128:(i % 16 + 1) * 128], ident[:], ident[:])

        def wchunk(layer, k, m):
            ci = W_BASE[layer] + 2 * k + m
            return w_sb[:, ci * 128:(ci + 1) * 128]

        def begin_tile(r0, rt, first):
            """Allocate input tile + issue its (prefetched) loads."""
            in_sb = inp.tile([128, 3 * r], BF16, tag="in")
            ng = 8 if first else 2
            hw_ = rt // ng
            for g in range(ng):
                for c in range(3):
                    nc.gpsimd.dma_start(
                        in_sb[:, c * rt + g * hw_:c * rt + (g + 1) * hw_],
                        in_d[c, :, r0 + g * hw_:r0 + (g + 1) * hw_])
            return {"r0": r0, "rt": rt, "in_sb": in_sb, "t": {}}

        def srcs_for(st, layer):
            rt = st["rt"]
            in_sb = st["in_sb"]
            xT = in_sb[:, 0:rt]
            hxT0 = in_sb[:, rt:2 * rt]
            hxT1 = in_sb[:, 2 * rt:3 * rt]
            if layer == "ff1":
                return [xT, hxT0, hxT1]
            if layer == "ic":
                return [xT]
            if layer == "rc":
                return [hxT0, hxT1]
            if layer == "ff2":
                f = st["t"]["ff1"]
                return [f[:, 0:rt], f[:, rt:2 * rt]]
            f = st["t"]["ff2"]
            return [f[:, 0:rt], f[:, rt:2 * rt]]

        FUNC = {"ff1": (AF.Gelu, 1.0), "ff2": (AF.Gelu, 1.0),
                "tab": (AF.Tanh, 0.5), "ic": (AF.Gelu, 1.0),
                "rc": (AF.Gelu, 1.0)}

        def emit_half(st, layer, m):
            """One psum group: matmuls for (layer, half m) + its ACTIVATE."""
            rt = st["rt"]
            if m == 0:
                st["t"][layer] = acts.tile(
                    [128, 2 * r], BF16, tag=layer, name=f"act_{layer}")
            out_tile = st["t"][layer]
            srcs = srcs_for(st, layer)
            func, scale = FUNC[layer]
            K = KCH[layer]
            col = BIAS_COL[(layer, m)]
            for h in range(rt // 2048):
                pt = ps.tile([128, 2048], F32, tag="mm")
                for k in range(K):
                    for j in range(4):
                        sl = slice(h * 2048 + j * 512, h * 2048 + (j + 1) * 512)
                        nc.tensor.matmul(
                            pt[:, j * 512:(j + 1) * 512],
                            wchunk(layer, k, m),
                            srcs[k][:, sl],
                            start=(k == 0), stop=(k == K - 1))
                nc.scalar.activation(
                    st["t"][layer][:, m * rt + h * 2048:m * rt + (h + 1) * 2048],
                    pt[:], func, bias=b_sb[:, col:col + 1], scale=scale)
            _ = out_tile

        def stage_b(st):
            """Combine on DVE, store feature-major via the DVE queue."""
            r0, rt = st["r0"], st["rt"]
            in_sb, u, ic, rc = st["in_sb"], st["t"]["tab"], st["t"]["ic"], st["t"]["rc"]
            o_sb = io.tile([128, 2 * r], BF16, tag="o")
            # out = hx + t*(ic+rc-hx);  t = 0.5*u + 0.5
            for m in range(2):
                msl = slice(m * rt, (m + 1) * rt)
                hxm = in_sb[:, (1 + m) * rt:(2 + m) * rt]
                ti = tmp.tile([128, r], BF16, tag="ti")
                ti = ti[:, 0:rt]
                nc.vector.tensor_scalar(ti, u[:, msl], 0.5, 0.5, ALU.mult, ALU.add)
                s = tmp.tile([128, r], BF16, tag="s")
                s = s[:, 0:rt]
                nc.vector.tensor_add(s, ic[:, msl], rc[:, msl])
                d = tmp.tile([128, r], BF16, tag="d")
                d = d[:, 0:rt]
                nc.vector.tensor_sub(d, s, hxm)
                p = tmp.tile([128, r], BF16, tag="p")
                p = p[:, 0:rt]
                nc.vector.tensor_mul(p, ti, d)
                nc.vector.tensor_add(o_sb[:, msl], p, hxm)
                nc.sync.dma_start(out_d[m, :, r0:r0 + rt], o_sb[:, msl])

        # layer-level software pipeline (see module docstring)
        cur = begin_tile(0, r, True)
        prev = None
        for i in range(nm):
            nxt = begin_tile((i + 1) * r, r, False) if i + 1 < nm else None
            bq = list(SEQ_B) if prev is not None else []
            for a_idx, (layer, m) in enumerate(SEQ_A):
                emit_half(cur, layer, m)
                if a_idx < len(bq):
                    emit_half(prev, *bq[a_idx])
            if prev is not None:
                stage_b(prev)
            prev, cur = cur, nxt
        for layer, m in SEQ_B:
            emit_half(prev, layer, m)
        stage_b(prev)
    nc.finalize()
    return nc


_NC_CACHE: dict = {}


def _get_nc(b_core: int, r: int) -> bass.Bass:
    key = (b_core, r)
    if key not in _NC_CACHE:
        _NC_CACHE[key] = build_nc(b_core, r)
    return _NC_CACHE[key]


def _prep_host(W_ff1, b_ff1, W_ff2, b_ff2, W_ta, b_ta, W_tb, b_tb,
               W_in, b_in, input_b, W_r, r_b):
    f32 = lambda a: np.asarray(a, dtype=np.float32)
    weights = {
        "ff1": f32(W_ff1),
        "ff2": f32(W_ff2),
        "tab": f32(W_ta) + f32(W_tb),
        "ic": f32(W_in),
        "rc": f32(W_r),
    }
    biases = {
        "ff1": f32(b_ff1),
        "ff2": f32(b_ff2),
        "tab": 0.5 * (f32(b_ta) + f32(b_tb)),
        "ic": f32(b_in) + f32(input_b),
        "rc": f32(r_b),
    }
    wstack = np.zeros([N_WCH, 128, 128], dtype=NP_BF16)
    for layer in LAYERS:
        W = weights[layer]
        for k in range(KCH[layer]):
            for m in range(2):
                ci = W_BASE[layer] + 2 * k + m
                wstack[ci] = np.ascontiguousarray(
                    W[m * 128:(m + 1) * 128, k * 128:(k + 1) * 128].T
                ).astype(NP_BF16)
    bstack = np.zeros([128, 10], dtype=np.float32)
    for li, layer in enumerate(LAYERS):
        for m in range(2):
            bstack[:, 2 * li + m] = biases[layer][m * 128:(m + 1) * 128]
    return wstack, bstack


def _run(inputs: dict, b_core: int = B_CORE, r: int = R, n_cores: int = N_CORES,
         **run_kwargs):
    x = np.asarray(inputs["x"], dtype=np.float32)
    hx = np.asarray(inputs["hx"], dtype=np.float32)
    wstack, bstack = _prep_host(
        inputs["W_ff1"], inputs["b_ff1"], inputs["W_ff2"], inputs["b_ff2"],
        inputs["W_ta"], inputs["b_ta"], inputs["W_tb"], inputs["b_tb"],
        inputs["W_in"], inputs["b_in"], inputs["input_b"], inputs["W_r"],
        inputs["r_b"])
    nc = _get_nc(b_core, r)
    xb = x.astype(NP_BF16)
    hxb = hx.astype(NP_BF16)
    in_maps = []
    for c in range(n_cores):
        sl = slice(c * b_core, (c + 1) * b_core)
        inT = np.empty((3, 128, b_core), dtype=NP_BF16)
        inT[0] = xb[sl].T
        inT[1] = hxb[sl, 0:128].T
        inT[2] = hxb[sl, 128:256].T
        in_maps.append({"inT": inT, "wstack": wstack, "bstack": bstack})
    res = run_bass_kernel_spmd(nc, in_maps, list(range(n_cores)), **run_kwargs)
    outs = []
    for m in res.results:
        o = m["outT"]  # [2, 128, b_core] bf16, feature-major
        outs.append(o.transpose(2, 0, 1).astype(np.float32).reshape(b_core, 256))
    out = np.concatenate(outs, axis=0)
    return out, res


def kernel(**inputs):
    out, _ = _run(inputs)
    return (out, out)


# revision 7
# speedup vs baseline: 1.0970x; 1.0477x over previous
"""Trainium2 Bass kernel for the CfC cell (nn_CfCCell), data-parallel on 8 cores.

Math (per row):
    ff1 = gelu(x_cat @ W_ff1.T + b_ff1)          x_cat = [x, hx]
    ff2 = gelu(ff1 @ W_ff2.T + b_ff2)
    t   = sigmoid(ff2 @ (W_ta+W_tb).T + b_ta+b_tb)      (TS == 1.0)
    ic  = gelu(x @ W_in.T + b_in + input_b)
    rc  = gelu(hx @ W_r.T + r_b)
    out = hx + t * (ic + rc - hx)

v3 design notes:
  * batch sharded 8 ways; all activations feature-major ([feat, batch]).
  * x/hx are cast to bf16 AND transposed on the HOST, so the device loads
    feature-major bf16 directly (halves HBM traffic, removes all PE
    transposes).  The output is stored feature-major bf16 and
    transposed/upcast on the host.
  * ScalarE is the critical engine (~157us of ACTIVATE work): PSUM is split
    4+4 banks, matmuls fill one [128,2048] f32 group while ScalarE drains
    the other with a single N=2048 ACTIVATE per layer-half (~87% efficient).
  * layer-level software pipeline: megatile i's independent layers
    (ic/rc/ff1) are interleaved with megatile i-1's dependent layers
    (ff2/tab) so ScalarE never stalls on the ff1->ff2->tab ACT chain.
  * input loads are prefetched one megatile ahead (in pool bufs=3); output
    stores ride the otherwise-idle SP queue so they never block the
    gpsimd load queue.
  * sigmoid via 0.5*tanh(z/2)+0.5 so every ScalarE op lives in the single
    "gelu_and_others" table set (no table reloads).
  * weights stay stationary across the 4 psum banks (k-outer matmul loop)
    to minimize LDWEIGHTS traffic.
"""

from contextlib import ExitStack

import ml_dtypes
import numpy as np

import concourse.bacc as bacc
import concourse.bass as bass
import concourse.mybir as mybir
import concourse.tile as tile
from concourse import masks
from concourse.bass_utils import run_bass_kernel_spmd

AF = mybir.ActivationFunctionType
ALU = mybir.AluOpType
BF16 = mybir.dt.bfloat16
F32 = mybir.dt.float32
NP_BF16 = ml_dtypes.bfloat16

B, I, H = 131072, 128, 256
N_CORES = 8
B_CORE = B // N_CORES  # 16384
R = 2048               # megatile rows (batch columns per megatile)

# layer order; K = contraction chunks of 128
LAYERS = ("ff1", "ff2", "tab", "ic", "rc")
KCH = {"ff1": 3, "ff2": 2, "tab": 2, "ic": 1, "rc": 2}
W_BASE = {}
_acc = 0
for _l in LAYERS:
    W_BASE[_l] = _acc
    _acc += KCH[_l] * 2
N_WCH = _acc  # 20 weight chunks of [128, 128]
BIAS_COL = {(_l, _m): 2 * _i + _m for _i, _l in enumerate(LAYERS) for _m in range(2)}

# per-megatile emission order: A = this megatile's input-only layers,
# B = previous megatile's ff2/tab (interleaved into A's slots)
SEQ_A = (("ic", 0), ("ic", 1), ("rc", 0), ("rc", 1), ("ff1", 0), ("ff1", 1))
SEQ_B = (("ff2", 0), ("ff2", 1), ("tab", 0), ("tab", 1))


def build_nc(b_core: int = B_CORE, r: int = R) -> bass.Bass:
    nm = b_core // r
    assert b_core % r == 0 and r % 1024 == 0

    nc = bacc.Bacc("TRN2")
    in_d = nc.dram_tensor("inT", [3, 128, b_core], BF16, kind="ExternalInput")
    w_d = nc.dram_tensor("wstack", [128, N_WCH * 128], BF16, kind="ExternalInput")
    b_d = nc.dram_tensor("bstack", [128, 10], F32, kind="ExternalInput")
    out_d = nc.dram_tensor("outT", [2, 128, b_core], BF16, kind="ExternalOutput")

    with tile.TileContext(nc) as tc, ExitStack() as ctx:
        const = ctx.enter_context(tc.tile_pool(name="const", bufs=1))
        w_sb = const.tile([128, N_WCH * 128], BF16)
        nc.sync.dma_start(w_sb[:], w_d[:])
        b_sb = const.tile([128, 10], F32)
        nc.sync.dma_start(b_sb[:], b_d[:])
        ident = const.tile([128, 128], BF16)
        masks.make_identity(nc, ident[:])

        inp = ctx.enter_context(tc.tile_pool(name="inp", bufs=3))
        io = ctx.enter_context(tc.tile_pool(name="io", bufs=2))
        acts3 = ctx.enter_context(tc.tile_pool(name="acts3", bufs=3))
        acts2 = ctx.enter_context(tc.tile_pool(name="acts2", bufs=2))
        tmp = ctx.enter_context(tc.tile_pool(name="tmp", bufs=1))
        ps = ctx.enter_context(tc.tile_pool(name="ps", bufs=2, space="PSUM"))

        # HAM warm-up: ~3.5us of dummy PE work while the first loads land, so
        # the first real matmuls run at 2.4 GHz instead of 1.2
        warm = ps.tile([128, 2048], F32, tag="mm")
        for i in range(32):
            nc.tensor.matmul(
                warm[:, (i % 16) * 128:(i % 16 + 1) * 128], ident[:], ident[:])
        warm2 = ps.tile([128, 2048], F32, tag="mm")
        for i in range(16):
            nc.tensor.matmul(
                warm2[:, (i % 16) * 128:(i % 16 + 1) * 128], ident[:], ident[:])

        def wchunk(layer, k, m):
            ci = W_BASE[layer] + 2 * k + m
            return w_sb[:, ci * 128:(ci + 1) * 128]

        def begin_tile(r0, rt, first):
            """Allocate input tile + issue its (prefetched) loads."""
            in_sb = inp.tile([128, 3 * r], BF16, tag="in")
            ng = 2
            hw_ = rt // ng
            for g in range(ng):
                for c in range(3):
                    nc.gpsimd.dma_start(
                        in_sb[:, c * rt + g * hw_:c * rt + (g + 1) * hw_],
                        in_d[c, :, r0 + g * hw_:r0 + (g + 1) * hw_])
            return {"r0": r0, "rt": rt, "in_sb": in_sb, "t": {}}

        def srcs_for(st, layer):
            rt = st["rt"]
            in_sb = st["in_sb"]
            xT = in_sb[:, 0:rt]
            hxT0 = in_sb[:, rt:2 * rt]
            hxT1 = in_sb[:, 2 * rt:3 * rt]
            if layer == "ff1":
                return [xT, hxT0, hxT1]
            if layer == "ic":
                return [xT]
            if layer == "rc":
                return [hxT0, hxT1]
            if layer == "ff2":
                f = st["t"]["ff1"]
                return [f[:, 0:rt], f[:, rt:2 * rt]]
            f = st["t"]["ff2"]
            return [f[:, 0:rt], f[:, rt:2 * rt]]

        FUNC = {"ff1": (AF.Gelu, 1.0), "ff2": (AF.Gelu, 1.0),
                "tab": (AF.Tanh, 0.5), "ic": (AF.Gelu, 1.0),
                "rc": (AF.Gelu, 1.0)}

        def emit_half(st, layer, m):
            """One psum group: matmuls for (layer, half m) + its ACTIVATE."""
            rt = st["rt"]
            if m == 0:
                pool = acts2 if layer in ("ff1", "ff2") else acts3
                st["t"][layer] = pool.tile(
                    [128, 2 * r], BF16, tag=layer, name=f"act_{layer}")
            out_tile = st["t"][layer]
            srcs = srcs_for(st, layer)
            func, scale = FUNC[layer]
            K = KCH[layer]
            col = BIAS_COL[(layer, m)]
            for h in range(rt // 2048):
                pt = ps.tile([128, 2048], F32, tag="mm")
                for k in range(K):
                    for j in range(4):
                        sl = slice(h * 2048 + j * 512, h * 2048 + (j + 1) * 512)
                        nc.tensor.matmul(
                            pt[:, j * 512:(j + 1) * 512],
                            wchunk(layer, k, m),
                            srcs[k][:, sl],
                            start=(k == 0), stop=(k == K - 1))
                nc.scalar.activation(
                    st["t"][layer][:, m * rt + h * 2048:m * rt + (h + 1) * 2048],
                    pt[:], func, bias=b_sb[:, col:col + 1], scale=scale)
            _ = out_tile

        def stage_b(st):
            """Combine on DVE, store feature-major via the DVE queue."""
            r0, rt = st["r0"], st["rt"]
            in_sb, u, ic, rc = st["in_sb"], st["t"]["tab"], st["t"]["ic"], st["t"]["rc"]
            o_sb = io.tile([128, 2 * r], BF16, tag="o")
            # out = hx + t*(ic+rc-hx);  t = 0.5*u + 0.5
            for m in range(2):
                msl = slice(m * rt, (m + 1) * rt)
                hxm = in_sb[:, (1 + m) * rt:(2 + m) * rt]
                ti = tmp.tile([128, r], BF16, tag="ti")
                ti = ti[:, 0:rt]
                nc.vector.tensor_scalar(ti, u[:, msl], 0.5, 0.5, ALU.mult, ALU.add)
                s = tmp.tile([128, r], BF16, tag="s")
                s = s[:, 0:rt]
                nc.vector.tensor_add(s, ic[:, msl], rc[:, msl])
                d = tmp.tile([128, r], BF16, tag="d")
                d = d[:, 0:rt]
                nc.vector.tensor_sub(d, s, hxm)
                p = tmp.tile([128, r], BF16, tag="p")
                p = p[:, 0:rt]
                nc.vector.tensor_mul(p, ti, d)
                nc.vector.tensor_add(o_sb[:, msl], p, hxm)
                nc.sync.dma_start(out_d[m, :, r0:r0 + rt], o_sb[:, msl])

        # layer-level software pipeline (see module docstring)
        cur = begin_tile(0, r, True)
        prev = None
        for i in range(nm):
            nxt = begin_tile((i + 1) * r, r, False) if i + 1 < nm else None
            bq = list(SEQ_B) if prev is not None else []
            for a_idx, (layer, m) in enumerate(SEQ_A):
                emit_half(cur, layer, m)
                if a_idx < len(bq):
                    emit_half(prev, *bq[a_idx])
            if prev is not None:
                stage_b(prev)
            prev, cur = cur, nxt
        for layer, m in SEQ_B:
            emit_half(prev, layer, m)
        stage_b(prev)
    nc.finalize()
    return nc


_NC_CACHE: dict = {}


def _get_nc(b_core: int, r: int) -> bass.Bass:
    key = (b_core, r)
    if key not in _NC_CACHE:
        _NC_CACHE[key] = build_nc(b_core, r)
    return _NC_CACHE[key]


def _prep_host(W_ff1, b_ff1, W_ff2, b_ff2, W_ta, b_ta, W_tb, b_tb,
               W_in, b_in, input_b, W_r, r_b):
    f32 = lambda a: np.asarray(a, dtype=np.float32)
    weights = {
        "ff1": f32(W_ff1),
        "ff2": f32(W_ff2),
        "tab": f32(W_ta) + f32(W_tb),
        "ic": f32(W_in),
        "rc": f32(W_r),
    }
    biases = {
        "ff1": f32(b_ff1),
        "ff2": f32(b_ff2),
        "tab": 0.5 * (f32(b_ta) + f32(b_tb)),
        "ic": f32(b_in) + f32(input_b),
        "rc": f32(r_b),
    }
    wstack = np.zeros([128, N_WCH * 128], dtype=NP_BF16)
    for layer in LAYERS:
        W = weights[layer]
        for k in range(KCH[layer]):
            for m in range(2):
                ci = W_BASE[layer] + 2 * k + m
                wstack[:, ci * 128:(ci + 1) * 128] = (
                    W[m * 128:(m + 1) * 128, k * 128:(k + 1) * 128].T
                ).astype(NP_BF16)
    bstack = np.zeros([128, 10], dtype=np.float32)
    for li, layer in enumerate(LAYERS):
        for m in range(2):
            bstack[:, 2 * li + m] = biases[layer][m * 128:(m + 1) * 128]
    return wstack, bstack


def _run(inputs: dict, b_core: int = B_CORE, r: int = R, n_cores: int = N_CORES,
         **run_kwargs):
    x = np.asarray(inputs["x"], dtype=np.float32)
    hx = np.asarray(inputs["hx"], dtype=np.float32)
    wstack, bstack = _prep_host(
        inputs["W_ff1"], inputs["b_ff1"], inputs["W_ff2"], inputs["b_ff2"],
        inputs["W_ta"], inputs["b_ta"], inputs["W_tb"], inputs["b_tb"],
        inputs["W_in"], inputs["b_in"], inputs["input_b"], inputs["W_r"],
        inputs["r_b"])
    nc = _get_nc(b_core, r)
    xb = x.astype(NP_BF16)
    hxb = hx.astype(NP_BF16)
    in_maps = []
    for c in range(n_cores):
        sl = slice(c * b_core, (c + 1) * b_core)
        inT = np.empty((3, 128, b_core), dtype=NP_BF16)
        inT[0] = xb[sl].T
        inT[1] = hxb[sl, 0:128].T
        inT[2] = hxb[sl, 128:256].T
        in_maps.append({"inT": inT, "wstack": wstack, "bstack": bstack})
    res = run_bass_kernel_spmd(nc, in_maps, list(range(n_cores)), **run_kwargs)
    outs = []
    for m in res.results:
        o = m["outT"]  # [2, 128, b_core] bf16, feature-major
        outs.append(o.transpose(2, 0, 1).astype(np.float32).reshape(b_core, 256))
    out = np.concatenate(outs, axis=0)
    return out, res


def kernel(**inputs):
    out, _ = _run(inputs)
    return (out, out)
